# revision 2
# baseline (speedup 1.0000x reference)
import sys

if "/opt/trn_rl_repo" not in sys.path:
    sys.path.insert(0, "/opt/trn_rl_repo")

import numpy as np
import ml_dtypes

from concourse import bass, tile, bacc
from concourse.bass import mybir

F32 = mybir.dt.float32
BF16 = mybir.dt.bfloat16

N_CORES = 8
N_TOTAL = 32768
N_CORE = N_TOTAL // N_CORES  # 4096 rows per core
D = 1024
C = 64
K = 16
DEPTH = 4
M = 1024
N_STAGES = 2
ROWS_STAGE = N_CORE // N_STAGES  # 2048
ALU = mybir.AluOpType
AFT = mybir.ActivationFunctionType


def build_program(dims, repeat=1):
    """dims: python list of 256 ints (compile-time gather indices)."""
    nc = bacc.Bacc()
    x_d = nc.declare_dram_parameter("x", [N_CORE, D], F32, isOutput=False)
    thr_d = nc.declare_dram_parameter("thrcols", [128, 15], F32, isOutput=False)
    lut_d = nc.declare_dram_parameter("lutT", [C * K, M], BF16, isOutput=False)
    kvec_d = nc.declare_dram_parameter("kvec", [128, 1], F32, isOutput=False)
    out_d = nc.declare_dram_parameter("out", [N_CORE, M], F32, isOutput=True)

    with tile.TileContext(nc) as tc:
        from contextlib import ExitStack
        es = ExitStack()
        pers = es.enter_context(tc.tile_pool(name="pers", bufs=1))

        def ptile(shape, dtype, name):
            return pers.tile(shape, dtype, name=name, tag=name)

        # ---- persistent tiles ----
        lutT = ptile([128, 8, M], BF16, "lutT_sb")       # [ck%128, j, m]
        thr = ptile([128, 15], F32, "thr_sb")
        kvec = ptile([128, 1], F32, "kvec_sb")
        ET = ptile([128, N_STAGES * 8, 2048], BF16, "ET_sb")
        xTS = ptile([128, 8, 32, 32], F32, "xTS_sb")     # [p, tau, u_hi, s]
        chosenT = ptile([128, DEPTH, 1024], F32, "chosenT_sb")
        bucketbf = ptile([128, 1024], BF16, "bucketbf_sb")
        tmps = [ptile([128, 1024], F32, f"tmp{ti}_sb") for ti in range(10)]
        b0, b1, b2, tA, tB, tC, tD, tE, tF, tG = tmps
        I8 = mybir.dt.int8
        b0i = ptile([128, 1024], I8, "b0i_sb")
        b1i = ptile([128, 1024], I8, "b1i_sb")

        xpool = es.enter_context(tc.tile_pool(name="xpool", bufs=2))
        opool = es.enter_context(tc.tile_pool(name="opool", bufs=2))
        pspool = es.enter_context(
            tc.tile_pool(name="pspool", bufs=2, space=bass.MemorySpace.PSUM)
        )

        nc.sync.dma_start(thr[:], thr_d[:])
        nc.sync.dma_start(kvec[:], kvec_d[:])
        for j in range(8):
            nc.sync.dma_start(lutT[:, j, :], lut_d[j * 128:(j + 1) * 128, :])

        # thr column APs
        def tcol(i):
            return thr[:, i:i + 1]

        for s in [s for _ in range(repeat) for s in range(N_STAGES)]:
            for hp in range(2):  # which 1024-row half of the stage
                # load + stream-transpose 8 x-tiles of this half
                for tau in range(8):
                    r0 = s * ROWS_STAGE + hp * 1024 + tau * 128
                    xt = xpool.tile([128, D], F32, name="xt", tag="xt")
                    nc.sync.dma_start(xt[:], x_d[r0:r0 + 128, :])
                    nc.vector.transpose(
                        xTS[:, tau].rearrange("p a b -> p (a b)"), xt[:]
                    )
                # row-copy DMAs: for each (c, d) pull column u into chosenT
                for c in range(C):
                    P = hp * 64 + c
                    for d in range(DEPTH):
                        u = dims[c * DEPTH + d]
                        src = xTS[u % 32::32, :, u // 32, :]          # [4, 8, 32]
                        dst = chosenT[P:P + 1, d, :].rearrange(
                            "p (bi t s) -> p bi t s", bi=4, t=8, s=32
                        )
                        nc.sync.dma_start(dst, src)

                # ---- descent on [128=(hp,c) x 1024] ----  (hp covers only one
                # half of partitions with fresh data per hp iteration; compute
                # once per stage after both halves are gathered)
            xd = [chosenT[:, d, :] for d in range(DEPTH)]
            nc.vector.tensor_scalar(b0[:], xd[0], tcol(0), None, ALU.is_gt)
            nc.vector.tensor_copy(b0i[:], b0[:])
            nc.vector.tensor_scalar(tA[:], b0[:], tcol(2), tcol(1), ALU.mult, ALU.add)
            nc.vector.tensor_tensor(b1[:], xd[1], tA[:], ALU.is_gt)
            nc.vector.tensor_copy(b1i[:], b1[:])

            nc.vector.tensor_scalar(tB[:], b1[:], tcol(4), tcol(3), ALU.mult, ALU.add)
            nc.vector.tensor_scalar(tC[:], b1[:], tcol(6), tcol(5), ALU.mult, ALU.add)
            nc.vector.tensor_copy(tA[:], tB[:])
            nc.vector.copy_predicated(tA[:], b0i[:], tC[:])
            nc.vector.tensor_tensor(b2[:], xd[2], tA[:], ALU.is_gt)

            nc.vector.tensor_scalar(tB[:], b2[:], tcol(8), tcol(7), ALU.mult, ALU.add)
            nc.vector.tensor_scalar(tC[:], b2[:], tcol(10), tcol(9), ALU.mult, ALU.add)
            nc.vector.tensor_scalar(tD[:], b2[:], tcol(12), tcol(11), ALU.mult, ALU.add)
            nc.vector.tensor_scalar(tE[:], b2[:], tcol(14), tcol(13), ALU.mult, ALU.add)
            nc.vector.tensor_copy(tF[:], tB[:])
            nc.vector.copy_predicated(tF[:], b1i[:], tC[:])
            nc.vector.tensor_copy(tG[:], tD[:])
            nc.vector.copy_predicated(tG[:], b1i[:], tE[:])
            nc.vector.tensor_copy(tA[:], tF[:])
            nc.vector.copy_predicated(tA[:], b0i[:], tG[:])
            nc.vector.tensor_tensor(tD[:], xd[3], tA[:], ALU.is_gt)   # b3 -> tD

            # bucket = 8*b0 + 4*b1 + 2*b2 + b3   (built as ((b0*2+b1)*2+b2)*2+b3)
            nc.vector.scalar_tensor_tensor(tB[:], b0[:], 2.0, b1[:], ALU.mult, ALU.add)
            nc.vector.scalar_tensor_tensor(tC[:], tB[:], 2.0, b2[:], ALU.mult, ALU.add)
            nc.vector.scalar_tensor_tensor(
                bucketbf[:], tC[:], 2.0, tD[:], ALU.mult, ALU.add
            )

            # ---- E^T: replicate bucket row to 8 partitions per k, compare ----
            for j in range(8):
                col = s * 8 + j
                for hp in range(2):
                    fsl = slice(hp * 1024, (hp + 1) * 1024)
                    nc.scalar.dma_start(
                        ET[0:8, col, fsl],
                        bucketbf[hp * 64 + 8 * j:hp * 64 + 8 * j + 8, :],
                    )
                    for dbl in range(4):
                        w = 8 << dbl
                        nc.scalar.dma_start(
                            ET[w:2 * w, col, fsl], ET[0:w, col, fsl]
                        )
                nc.vector.tensor_scalar(
                    ET[:, col, :], ET[:, col, :], kvec[:], None,
                    ALU.is_equal,
                )

            # ---- matmul + output ----
            for i in range(16):
                ps = [
                    pspool.tile([128, 512], F32, name=f"ps{mc}", tag=f"ps{mc}")
                    for mc in range(2)
                ]
                for j in range(8):
                    lhsT = ET[:, s * 8 + j, i * 128:(i + 1) * 128]
                    for mc in range(2):
                        nc.tensor.matmul(
                            ps[mc][:], lhsT, lutT[:, j, mc * 512:(mc + 1) * 512],
                            start=(j == 0), stop=(j == 7),
                        )
                osb = opool.tile([128, M], F32, name="osb", tag="osb")
                nc.scalar.activation(osb[:, 0:512], ps[0][:], AFT.Copy)
                nc.scalar.activation(osb[:, 512:1024], ps[1][:], AFT.Copy)

                ih = i % 8
                hp = i // 8
                base = s * ROWS_STAGE + hp * 1024 + (ih % 2) * 512 + (ih // 2) * 32
                dview = out_d[:].rearrange("(a b c) m -> a b c m", b=4, c=32)
                a0 = base // 128
                nc.sync.dma_start(dview[a0:a0 + 4, (base % 128) // 32, :, :], osb[:])
        es.close()
    nc.finalize()
    return nc


def _prep_inputs(inputMatrix, dims, thresholds, lut):
    x = np.ascontiguousarray(np.asarray(inputMatrix, dtype=np.float32))
    dims = [int(v) for v in np.asarray(dims).ravel()]
    thr = np.asarray(thresholds, dtype=np.float32).reshape(C, K - 1)
    lut = np.asarray(lut, dtype=np.float32)

    # thrcols [128, 15]: t0,t1,d21,t3,d43,t5,d65,t7,d87,t9,d109,t11,d1211,t13,d1413
    tcols = np.empty((C, 15), dtype=np.float32)
    tcols[:, 0] = thr[:, 0]
    pairs = [(1, 2), (3, 4), (5, 6), (7, 8), (9, 10), (11, 12), (13, 14)]
    for idx, (lo, hi) in enumerate(pairs):
        tcols[:, 1 + 2 * idx] = thr[:, lo]
        tcols[:, 2 + 2 * idx] = thr[:, hi] - thr[:, lo]
    thrcols = np.concatenate([tcols, tcols], axis=0)  # [128, 15]

    # lutT [j*128 + k*8 + c_loc, m] = lut[m, 8j + c_loc, k]
    lt = lut.reshape(M, 8, 8, K).transpose(1, 3, 2, 0).reshape(C * K, M)
    lutT = lt.astype(ml_dtypes.bfloat16)

    kvec = (np.arange(128) // 8).astype(np.float32).reshape(128, 1)
    return x, dims, thrcols, lutT, kvec


def prep_run(inputs):
    x, dims_l, thrcols, lutT, kvec = _prep_inputs(
        inputs["inputMatrix"], inputs["dims"], inputs["thresholds"], inputs["lut"]
    )
    nc = build_program(dims_l)
    in_maps = [
        {
            "x": np.ascontiguousarray(x[i * N_CORE:(i + 1) * N_CORE]),
            "thrcols": thrcols,
            "lutT": lutT,
            "kvec": kvec,
        }
        for i in range(N_CORES)
    ]
    return in_maps, nc


def kernel(inputMatrix, dims, thresholds, lut, selection_matrix=None,
           tree_des_mat=None):
    from concourse.bass_utils import run_bass_kernel_spmd

    in_maps, nc = prep_run({
        "inputMatrix": inputMatrix, "dims": dims,
        "thresholds": thresholds, "lut": lut,
    })
    res = run_bass_kernel_spmd(nc, in_maps, list(range(N_CORES)))
    out = np.concatenate(
        [np.asarray(res.results[i]["out"]) for i in range(N_CORES)], axis=0
    )
    return out.astype(np.float32)



# revision 4
# speedup vs baseline: 3.1680x; 3.1680x over previous
import sys

if "/opt/trn_rl_repo" not in sys.path:
    sys.path.insert(0, "/opt/trn_rl_repo")

import numpy as np
import ml_dtypes

from concourse import bass, tile, bacc, library_config, masks
from concourse.bass import mybir

F32 = mybir.dt.float32
BF16 = mybir.dt.bfloat16
I16 = mybir.dt.int16
I8 = mybir.dt.int8

N_CORES = 8
N_TOTAL = 32768
N_CORE = N_TOTAL // N_CORES  # 4096 rows per core
D = 1024
C = 64
K = 16
DEPTH = 4
M = 1024
ROWS_STAGE = 1024  # 8 tiles of 128 rows per stage
ALU = mybir.AluOpType
AFT = mybir.ActivationFunctionType


def build_program(dims, n_rows=N_CORE):
    """dims: python list of 256 ints (compile-time gather indices; unused —
    gather indices travel via the idx input tensor)."""
    n_stages = n_rows // ROWS_STAGE
    nc = bacc.Bacc()
    x_d = nc.declare_dram_parameter("x", [n_rows, D], F32, isOutput=False)
    thr_d = nc.declare_dram_parameter("thrrep", [128, 15 * 512], F32, isOutput=False)
    lut_d = nc.declare_dram_parameter("lutT", [C * K, M], BF16, isOutput=False)
    kvec_d = nc.declare_dram_parameter("kvec", [128, 1], F32, isOutput=False)
    idx_d = nc.declare_dram_parameter("idx", [128, 32], I16, isOutput=False)
    out_d = nc.declare_dram_parameter("out", [n_rows, M], BF16, isOutput=True)

    with tile.TileContext(nc) as tc:
        from contextlib import ExitStack
        es = ExitStack()
        pers = es.enter_context(tc.tile_pool(name="pers", bufs=1))

        def ptile(shape, dtype, name):
            return pers.tile(shape, dtype, name=name, tag=name)

        # ---- persistent tiles ----
        lutT = ptile([128, 8, M], BF16, "lutT_sb")        # [(k*8+q), j, m]
        thrrep = ptile([128, 15, 8, 64], F32, "thr_sb")   # [p, node, t, c]
        kvec = ptile([128, 1], F32, "kvec_sb")            # k = p // 8
        idx = ptile([128, 32], I16, "idx_sb")
        ident = ptile([128, 128], BF16, "ident_sb")

        nc.sync.dma_start(thrrep[:].rearrange("p a b c -> p (a b c)"), thr_d[:])
        nc.sync.dma_start(kvec[:], kvec_d[:])
        nc.sync.dma_start(idx[:], idx_d[:])
        for j in range(8):
            nc.sync.dma_start(lutT[:, j, :], lut_d[j * 128:(j + 1) * 128, :])
        masks.make_identity(nc, ident[:])
        nc.gpsimd.load_library(library_config.ap_gather)

        # descent temporaries (serial on DVE, single-buffered)
        tmps = [ptile([128, 8, 64], F32, f"tmp{i}_sb") for i in range(8)]
        b0, b1, b2, b3, tA, tB, tC, tD = tmps
        b0i = ptile([128, 8, 64], I8, "b0i_sb")
        b1i = ptile([128, 8, 64], I8, "b1i_sb")

        xpool = es.enter_context(tc.tile_pool(name="xpool", bufs=3))
        chpool = es.enter_context(tc.tile_pool(name="chpool", bufs=2))
        bkpool = es.enter_context(tc.tile_pool(name="bkpool", bufs=2))
        btpool = es.enter_context(tc.tile_pool(name="btpool", bufs=2))
        reppool = es.enter_context(tc.tile_pool(name="reppool", bufs=2))
        etpool = es.enter_context(tc.tile_pool(name="etpool", bufs=2))
        opool = es.enter_context(tc.tile_pool(name="opool", bufs=2))
        pspool = es.enter_context(
            tc.tile_pool(name="pspool", bufs=2, space=bass.MemorySpace.PSUM)
        )
        ptpool = es.enter_context(
            tc.tile_pool(name="ptpool", bufs=2, space=bass.MemorySpace.PSUM)
        )

        def T(i):
            return thrrep[:, i]

        for s in range(n_stages):
            r0 = s * ROWS_STAGE
            # ---- load + gather: chosen[p, t, cd] = x[r0+t*128+p, dims[cd]]
            ch = chpool.tile([128, 8, 256], F32, name="ch", tag="ch")
            for a in range(4):
                xt = xpool.tile([128, 2, D], F32, name="xt", tag="xt")
                nc.sync.dma_start(
                    xt[:],
                    x_d[r0 + a * 256:r0 + (a + 1) * 256, :].rearrange(
                        "(a p) d -> p a d", a=2
                    ),
                )
                nc.gpsimd.ap_gather(
                    ch[:, 2 * a:2 * a + 2, :].rearrange("p a b -> p (a b)"),
                    xt[:].rearrange("p a b -> p (a b)"),
                    idx[:],
                    channels=128, num_elems=2 * D, d=1, num_idxs=512,
                )

            # ---- tree descent on [128, 8, 64] views ----
            x0 = ch[:, :, 0::4]
            x1 = ch[:, :, 1::4]
            x2 = ch[:, :, 2::4]
            x3 = ch[:, :, 3::4]
            TT = nc.vector.tensor_tensor
            nc.vector.tensor_tensor(b0[:], x0, T(0), ALU.is_gt)
            nc.vector.tensor_copy(b0i[:], b0[:])
            TT(tA[:], b0[:], T(2), ALU.mult)
            TT(tA[:], tA[:], T(1), ALU.add)
            TT(b1[:], x1, tA[:], ALU.is_gt)
            nc.vector.tensor_copy(b1i[:], b1[:])

            TT(tA[:], b1[:], T(4), ALU.mult)
            TT(tA[:], tA[:], T(3), ALU.add)
            TT(tB[:], b1[:], T(6), ALU.mult)
            TT(tB[:], tB[:], T(5), ALU.add)
            nc.vector.copy_predicated(tA[:], b0i[:], tB[:])
            TT(b2[:], x2, tA[:], ALU.is_gt)

            TT(tA[:], b2[:], T(8), ALU.mult)
            TT(tA[:], tA[:], T(7), ALU.add)
            TT(tB[:], b2[:], T(10), ALU.mult)
            TT(tB[:], tB[:], T(9), ALU.add)
            nc.vector.copy_predicated(tA[:], b1i[:], tB[:])
            TT(tC[:], b2[:], T(12), ALU.mult)
            TT(tC[:], tC[:], T(11), ALU.add)
            TT(tD[:], b2[:], T(14), ALU.mult)
            TT(tD[:], tD[:], T(13), ALU.add)
            nc.vector.copy_predicated(tC[:], b1i[:], tD[:])
            nc.vector.copy_predicated(tA[:], b0i[:], tC[:])
            TT(b3[:], x3, tA[:], ALU.is_gt)

            bucket = bkpool.tile([128, 8, 64], BF16, name="bucket", tag="bucket")
            nc.vector.scalar_tensor_tensor(tB[:], b0[:], 2.0, b1[:], ALU.mult, ALU.add)
            nc.vector.scalar_tensor_tensor(tC[:], tB[:], 2.0, b2[:], ALU.mult, ALU.add)
            nc.vector.scalar_tensor_tensor(
                bucket[:], tC[:], 2.0, b3[:], ALU.mult, ALU.add
            )

            # ---- transpose bucket to [c, n] via PE, evac via scalar ----
            bucketT = btpool.tile([64, ROWS_STAGE], BF16, name="bucketT", tag="bucketT")
            for t in range(8):
                pst = ptpool.tile([64, 128], BF16, name="pst", tag="pst")
                nc.tensor.transpose(pst[:], bucket[:, t, :], ident[:])
                nc.scalar.activation(
                    bucketT[:, t * 128:(t + 1) * 128], pst[:], AFT.Copy
                )

            # ---- replicate c -> (k, q) via seed + doubling DMAs ----
            rep = reppool.tile([128, 8, ROWS_STAGE], BF16, name="rep", tag="rep")
            for j in range(8):
                nc.scalar.dma_start(rep[0:8, j, :], bucketT[8 * j:8 * j + 8, :])
            w = 8
            while w < 128:
                nc.scalar.dma_start(rep[w:2 * w, :, :], rep[0:w, :, :])
                w *= 2

            # ---- one-hot: ET[p, j, n] = (rep == k(p)) ----
            ET = etpool.tile([128, 8, ROWS_STAGE], BF16, name="ET", tag="ET")
            nc.vector.tensor_scalar(ET[:], rep[:], kvec[:], None, ALU.is_equal)

            # ---- matmul + output ----
            for t in range(8):
                ps = [
                    pspool.tile([128, 512], F32, name=f"ps{mc}", tag=f"ps{mc}")
                    for mc in range(2)
                ]
                for j in range(8):
                    lhsT = ET[:, j, t * 128:(t + 1) * 128]
                    for mc in range(2):
                        nc.tensor.matmul(
                            ps[mc][:], lhsT, lutT[:, j, mc * 512:(mc + 1) * 512],
                            start=(j == 0), stop=(j == 7),
                        )
                if t % 2 == 0:
                    osb = opool.tile([128, 2, M], BF16, name="osb", tag="osb")
                nc.scalar.activation(osb[:, t % 2, 0:512], ps[0][:], AFT.Copy)
                nc.scalar.activation(osb[:, t % 2, 512:1024], ps[1][:], AFT.Copy)
                if t % 2 == 1:
                    rr = r0 + (t - 1) * 128
                    nc.sync.dma_start(
                        out_d[rr:rr + 256, :].rearrange("(a p) m -> p a m", a=2),
                        osb[:],
                    )
        es.close()
    nc.finalize()
    return nc


def _prep_inputs(inputMatrix, dims, thresholds, lut):
    x = np.ascontiguousarray(np.asarray(inputMatrix, dtype=np.float32))
    dims = [int(v) for v in np.asarray(dims).ravel()]
    thr = np.asarray(thresholds, dtype=np.float32).reshape(C, K - 1)
    lut = np.asarray(lut, dtype=np.float32)

    # threshold table [15, C]: t0, t1, d21, t3, d43, t5, d65, t7, d87, t9,
    # d109, t11, d1211, t13, d1413
    tbl = np.empty((15, C), dtype=np.float32)
    tbl[0] = thr[:, 0]
    pairs = [(1, 2), (3, 4), (5, 6), (7, 8), (9, 10), (11, 12), (13, 14)]
    for i, (lo, hi) in enumerate(pairs):
        tbl[1 + 2 * i] = thr[:, lo]
        tbl[2 + 2 * i] = thr[:, hi] - thr[:, lo]
    thrrep = np.ascontiguousarray(
        np.broadcast_to(tbl[None, :, None, :], (128, 15, 8, 64))
    ).reshape(128, 15 * 512)

    # lutT [j*128 + k*8 + q, m] = lut[m, 8j + q, k]
    lt = lut.reshape(M, 8, 8, K).transpose(1, 3, 2, 0).reshape(C * K, M)
    lutT = lt.astype(ml_dtypes.bfloat16)

    kvec = (np.arange(128) // 8).astype(np.float32).reshape(128, 1)

    # gather index list for a [128, 2, 1024]-flattened pair of row tiles
    lst = np.array(dims + [D + u for u in dims], dtype=np.int16)  # 512 idxs
    idx16 = np.zeros((16, 32), dtype=np.int16)
    for j, u in enumerate(lst):
        idx16[j % 16, j // 16] = u
    idx = np.ascontiguousarray(np.tile(idx16, (8, 1)))
    return x, dims, thrrep, lutT, kvec, idx


def prep_run(inputs):
    x, dims_l, thrrep, lutT, kvec, idx = _prep_inputs(
        inputs["inputMatrix"], inputs["dims"], inputs["thresholds"], inputs["lut"]
    )
    nc = build_program(dims_l)
    in_maps = [
        {
            "x": np.ascontiguousarray(x[i * N_CORE:(i + 1) * N_CORE]),
            "thrrep": thrrep,
            "lutT": lutT,
            "kvec": kvec,
            "idx": idx,
        }
        for i in range(N_CORES)
    ]
    return in_maps, nc


def kernel(inputMatrix, dims, thresholds, lut, selection_matrix=None,
           tree_des_mat=None):
    from concourse.bass_utils import run_bass_kernel_spmd

    in_maps, nc = prep_run({
        "inputMatrix": inputMatrix, "dims": dims,
        "thresholds": thresholds, "lut": lut,
    })
    res = run_bass_kernel_spmd(nc, in_maps, list(range(N_CORES)))
    out = np.concatenate(
        [np.asarray(res.results[i]["out"]) for i in range(N_CORES)], axis=0
    )
    return out.astype(np.float32)
